# revision 2
# baseline (speedup 1.0000x reference)
"""Trainium2 Bass kernel for the CapsuleLayer routing problem.

Strategy: shard in_nodes (i) across the 8 cores; each core owns 144 input
capsules. Per routing iteration the partial s[b,(j,d)] is a 1152-deep
bf16 matmul, summed across cores with one AllReduce; the b-logit update
(P = x^T v, W.P Hadamard + d-reduce, blocked-ones k-sum matmul, softmax,
Wc = W*c) and the next s-matmul are software-pipelined over 3 super-tiles
of 3 ik-tiles so Tensor/Vector/Scalar overlap. The last iteration uses a
ReduceScatter and each core squashes + writes only its own batch slice.

Perf notes (from ntff profiles; ~117us typical, was 141.6us baseline):
  - All matmuls bf16 with the natural jd=160 moving dim: bf16 streams
    1 cyc/row at any width (f32r needed a 256 pad and ran ~1.6x slower).
  - Collective payloads fp16, laid out partition-major for the two ARs
    (row p = batches p,p+128) so DRAM<->SBUF staging is contiguous; the
    RS also runs partition-major, core c's slice is batches {16c..} and
    {128+16c..}, re-interleaved on the host for free.
  - PSUM accumulation-chain rule: a chain's start marks its whole 2KB
    bank pending-zero, so concurrently-open chains must live in separate
    banks (s_ps strides 512 f32) and the P psum is double-buffered by
    super-tile parity to not stall behind the W.P Hadamard read.
  - Consecutive CC ops do NOT pipeline (measured: 2 half ARs take 1.7x
    one full AR), and the CC firmware floor is ~9-15us per op.
  - Engine clocks throttle ~20% when hot; A/B timings must be compared
    via the median MATMUL duration (337ns full clock).
  - Exec time is gated by core-dispatch skew (an 16-92us pre-AR0 barrier
    absorbs it); everything after runs lockstep: AR0 + 2x ~17us gaps +
    AR1 + RS ~= 71.5us chain.
"""
import sys

for _p in ("/opt/trn_rl_repo",):
    if _p not in sys.path:
        sys.path.insert(0, _p)

import numpy as np
import ml_dtypes

import concourse.bass as bass
import concourse.bacc as bacc
import concourse.mybir as mybir
import concourse.tile as tile
from concourse.bass_utils import run_bass_kernel_spmd

F32 = mybir.dt.float32
BF16 = mybir.dt.bfloat16
F16 = mybir.dt.float16
AF = mybir.ActivationFunctionType
ALU = mybir.AluOpType

IN_NODES, OUT_NODES = 1152, 10
IN_DIM, OUT_DIM = 8, 16
B = 256
N_CORES = 8
ITERS = 3
I_LOC = IN_NODES // N_CORES          # 144
IK = I_LOC * IN_DIM                  # 1152
NT = IK // 128                       # 9 sbuf tiles over the (i,k) axis
JD = OUT_NODES * OUT_DIM             # 160
JDS = 256                            # psum stride per pp tile (1KB sub-bank chunks)
SBS = 512                            # s_ps per-half stride: one full 2KB bank each
B_LOC = B // N_CORES                 # 32
RG = [list(range(N_CORES))]
CC_DT = F16                          # collective payload dtype
Y_DT = BF16                          # b-logit update path dtype
SPLIT_CC = False                     # split-AR probe: CC ops don't pipeline
TPG = 3                              # ik-tiles per super-tile
NG = NT // TPG                       # 3 super-tiles


def build_nc():
    nc = bacc.Bacc(
        "TRN2",
        target_bir_lowering=False,
        debug=False,
        enable_asserts=False,
        num_devices=N_CORES,
    )
    # partition-major layouts: per-partition data is contiguous, so each
    # chunked load is one descriptor per partition row instead of one per
    # (tile, partition) pair
    xT_d = nc.dram_tensor("xT", [128, NT, B], BF16, kind="ExternalInput")
    xb_d = nc.dram_tensor("xb", [128, 2, IK], BF16, kind="ExternalInput")
    wb_d = nc.dram_tensor("wb", [128, NT, JD], BF16, kind="ExternalInput")
    ones_d = nc.dram_tensor("onesb", [128, 128], BF16, kind="ExternalInput")
    out_d = nc.dram_tensor("out", [B_LOC, JD], F32, kind="ExternalOutput")

    with tile.TileContext(nc) as tc:
        with (
            tc.tile_pool(name="big", bufs=1) as bigp,
            tc.tile_pool(name="work", bufs=2) as workp,
            tc.tile_pool(name="psum", bufs=1, space="PSUM") as psum,
            tc.tile_pool(name="dram", bufs=2, space="DRAM") as dramp,
        ):
            W_sb = bigp.tile([128, NT, JD], BF16)
            Wc_sb = bigp.tile([128, NT, JD], BF16)
            xT_sb = bigp.tile([128, NT * B], BF16)       # (128, 2304)
            x_sb = bigp.tile([128, 2 * IK], BF16)        # (128, 2304)
            ones_sb = bigp.tile([128, 128], BF16)
            b_sb = bigp.tile([128, NT, OUT_NODES], F32)  # logits
            v_sb = bigp.tile([128, 2, JD], BF16)

            nc.gpsimd.dma_start(ones_sb[:], ones_d[:])
            # W on sync, xT on scalar, leading with single-tile chunks so
            # the first s-matmul links start after ~100KB instead of a
            # third of the bytes
            xT_v = xT_sb[:].rearrange("p (t b) -> p t b", b=B)
            for a, b in ((0, 1), (1, 3), (3, 6), (6, 9)):
                nc.sync.dma_start(W_sb[:, a:b, :], wb_d[:, a:b, :])
                nc.scalar.dma_start(xT_v[:, a:b, :], xT_d[:, a:b, :])
            h_xb = nc.gpsimd.dma_start(
                x_sb[:].rearrange("p (g i) -> p g i", i=IK),
                xb_d[:])
            nc.gpsimd.memset(b_sb[:], 0.0)
            # prime both ACT tables (Exp then Sqrt, so Sqrt is resident for
            # the first squash) off the critical path
            tprime = workp.tile([128, 8], F32, tag="tprime")
            nc.scalar.activation(tprime[:], b_sb[:, 0, 0:8], AF.Exp)
            nc.scalar.activation(tprime[:], b_sb[:, 0, 0:8], AF.Sqrt)

            s_ps = psum.tile([128, 2, SBS], F32, tag="s_ps", bufs=1)
            for it in range(ITERS):
                # ---- s-matmul: s[b, (j,d)] partial over local i ----
                # (for it>0 the matmuls are issued inside the super-tile
                # pipeline below; it==0 issues them here)
                if it == 0:
                    for g in range(NG):
                        for t in range(g * TPG, (g + 1) * TPG):
                            for b0 in range(2):
                                nc.tensor.matmul(
                                    s_ps[:, b0, 0:JD],
                                    xT_sb[:, t * B + b0 * 128:
                                          t * B + b0 * 128 + 128],
                                    W_sb[:, t, :],
                                    start=(t == 0),
                                    stop=(t == NT - 1),
                                )
                # stage CC input as fp16, one half per engine/queue.
                # AR iterations use a partition-major DRAM layout (row p =
                # batches p and p+128) so the DRAM<->SBUF DMAs are 128
                # contiguous 640B rows; AllReduce is elementwise so the
                # layout is free. The RS iteration needs batch-major.
                s16 = workp.tile([128, 2, JD], CC_DT, tag="s16")
                nc.scalar.copy(s16[:, 0, :], s_ps[:, 0, 0:JD])
                if it == 0:
                    # vector is idle before the first collective
                    nc.vector.tensor_copy(s16[:, 1, :], s_ps[:, 1, 0:JD])
                else:
                    nc.scalar.copy(s16[:, 1, :], s_ps[:, 1, 0:JD])
                if it < ITERS - 1 and SPLIT_CC:
                    sins = []
                    for h in range(2):
                        si = dramp.tile([128, JD], CC_DT, tag=f"cc_in{h}")
                        sins.append(si)
                    h_sin = nc.sync.dma_start(sins[0][:], s16[:, 0, :])
                    nc.sync.dma_start(sins[1][:], s16[:, 1, :])
                elif it < ITERS - 1:
                    sin = dramp.tile([128, 2 * JD], CC_DT, tag="cc_in")
                    sin_v = sin[:].rearrange("p (g j) -> p g j", j=JD)
                    h_sin = nc.sync.dma_start(sin_v[:], s16[:])
                else:
                    # partition-major for the RS too: core c's output slice
                    # is rows 16c:16c+16 = batches {16c..} and {128+16c..};
                    # the host re-interleaves (free).
                    sin = dramp.tile([128, 2 * JD], CC_DT, tag="cc_in_rs")
                    sin_v = sin[:].rearrange("p (g j) -> p g j", j=JD)
                    h_sin = nc.sync.dma_start(sin_v[:], s16[:])
                if it == 0:
                    # keep the x load off the critical DMA path
                    bass._add_dep_helper(
                        h_xb.ins, h_sin.ins, sync=True,
                        reason="defer x load until s staged")

                if it < ITERS - 1:
                    s_sb = workp.tile([128, 2, JD], CC_DT, tag="s_sb")
                    if SPLIT_CC:
                        souts = []
                        for h in range(2):
                            so = dramp.tile([128, JD], CC_DT,
                                            tag=f"cc_out{h}",
                                            addr_space="Shared")
                            nc.gpsimd.collective_compute(
                                "AllReduce", ALU.add, replica_groups=RG,
                                ins=[sins[h][:]],
                                outs=[so[:]],
                            )
                            souts.append(so)
                        nc.sync.dma_start(s_sb[:, 0, :], souts[0][:])
                        nc.sync.dma_start(s_sb[:, 1, :], souts[1][:])
                    else:
                        sout = dramp.tile([128, 2 * JD], CC_DT, tag="cc_out",
                                          addr_space="Shared")
                        nc.gpsimd.collective_compute(
                            "AllReduce", ALU.add, replica_groups=RG,
                            ins=[sin[:]], outs=[sout[:]],
                        )
                        sout_v = sout[:].rearrange("p (g j) -> p g j", j=JD)
                        nc.sync.dma_start(s_sb[:], sout_v[:])
                    # ---- squash per half: v = s * f, f = sqrt(sq)/(1+sq)
                    # iteration 0 runs on raw W (c is uniform 1/10): the
                    # 0.01 is folded into the Sqrt activation scale (giving
                    # 0.1*sqrt(sq)) and into den = 1 + 0.01*sq.
                    # h1's square+reduce run on gpsimd so h0's critical
                    # path to the first P-matmul isn't queued behind them.
                    f = workp.tile([128, 2, OUT_NODES], F32, tag="f")
                    c01 = 0.01 if it == 0 else 1.0
                    sq2 = [None, None]
                    for h in range(2):
                        ssq = workp.tile([128, JD], F32, tag=f"ssq{h}")
                        nc.vector.tensor_tensor(
                            ssq[:], s_sb[:, h, :], s_sb[:, h, :], op=ALU.mult)
                        sq = workp.tile([128, OUT_NODES], F32, tag=f"sq{h}")
                        nc.vector.tensor_reduce(
                            sq[:],
                            ssq[:].rearrange("p (j d) -> p j d", d=OUT_DIM),
                            axis=mybir.AxisListType.X, op=ALU.add,
                        )
                        sq2[h] = sq
                    for h in range(2):
                        sq = sq2[h]
                        rt = workp.tile([128, OUT_NODES], F32, tag=f"rt{h}")
                        # rt = c01 * sqrt(sq): the extra c01 accounts for
                        # v = squash(c01^0.5-scaled s) applied to raw s
                        nc.scalar.activation(rt[:], sq[:], AF.Sqrt,
                                             scale=c01 * c01)
                        den = workp.tile([128, OUT_NODES], F32, tag=f"den{h}")
                        nc.vector.tensor_scalar(den[:], sq[:], c01, 1.0,
                                                op0=ALU.mult, op1=ALU.add)
                        dri = workp.tile([128, OUT_NODES], F32, tag=f"dri{h}")
                        nc.vector.reciprocal(dri[:], den[:])
                        nc.vector.tensor_tensor(f[:, h, :], rt[:], dri[:],
                                                op=ALU.mult)
                        f_b = (f[:, h, :].unsqueeze(2)
                               .broadcast_to([128, OUT_NODES, OUT_DIM]))
                        nc.vector.tensor_tensor(
                            v_sb[:, h, :].rearrange("p (j d) -> p j d",
                                                    d=OUT_DIM),
                            s_sb[:, h, :].rearrange("p (j d) -> p j d",
                                                    d=OUT_DIM),
                            f_b, op=ALU.mult,
                        )
                    # ---- super-tile pipeline: P, z, y, k-sum, b, c, Wc,
                    # and the next iteration's s-matmul ----
                    y_ps = psum.tile([128, NG, 32], F32, tag="y_ps",
                                     bufs=1)
                    # P-matmuls first, with each super-tile's W.P Hadamard
                    # and d-reduce (vector) issued right behind its 6 links
                    # so they start as soon as that tile lands. The P psum
                    # is double-buffered by super-tile parity: a chain's
                    # `start` conceptually zeroes its whole bank, so with a
                    # single buffer super-tile g+1's matmuls would stall
                    # behind z_g's read of the shared bank.
                    pps = []
                    for g in range(NG):
                        pp_g = psum.tile([128, TPG, JDS], F32, tag="pp_ps",
                                         bufs=2)
                        for ti in range(TPG):
                            t = g * TPG + ti
                            for b0 in range(2):
                                nc.tensor.matmul(
                                    pp_g[:, ti, 0:JD],
                                    x_sb[:, b0 * IK + t * 128:
                                         b0 * IK + t * 128 + 128],
                                    v_sb[:, b0, :],
                                    start=(b0 == 0),
                                    stop=(b0 == 1),
                                )
                        pps.append(pp_g)
                    for g in range(NG):
                        gs = slice(g * TPG, (g + 1) * TPG)
                        z_g = workp.tile([128, TPG, JD], BF16, tag=f"z{g}")
                        nc.vector.tensor_tensor(
                            z_g[:], W_sb[:, gs, :], pps[g][:, :, 0:JD],
                            op=ALU.mult,
                        )
                        y_g = workp.tile([128, TPG, OUT_NODES], Y_DT,
                                         tag=f"y{g}")
                        with nc.allow_low_precision(
                                reason="b-logit update tolerates bf16"):
                            nc.vector.tensor_reduce(
                                y_g[:],
                                z_g[:].rearrange("p t (j d) -> p t j d",
                                                 d=OUT_DIM),
                                axis=mybir.AxisListType.X, op=ALU.add,
                            )
                        # k-sum + broadcast over k + 1/B scale in one matmul
                        nc.tensor.matmul(
                            y_ps[:, g, 0:TPG * OUT_NODES],
                            ones_sb[:],
                            y_g[:].rearrange("p t j -> p (t j)"),
                            start=True, stop=True)
                        # b += upd ; c = softmax(b) ; Wc = W * c
                        nc.vector.tensor_tensor(
                            b_sb[:, gs, :], b_sb[:, gs, :],
                            y_ps[:, g, 0:TPG * OUT_NODES].rearrange(
                                "p (t j) -> p t j", j=OUT_NODES),
                            op=ALU.add)
                        e_g = workp.tile([128, TPG, OUT_NODES], F32,
                                         tag=f"e{g}")
                        nc.scalar.activation(e_g[:], b_sb[:, gs, :], AF.Exp)
                        ds_g = workp.tile([128, TPG], F32, tag=f"ds{g}")
                        nc.vector.tensor_reduce(
                            ds_g[:], e_g[:],
                            axis=mybir.AxisListType.X, op=ALU.add,
                        )
                        r_g = workp.tile([128, TPG], F32, tag=f"r{g}")
                        nc.vector.reciprocal(r_g[:], ds_g[:])
                        c_g = workp.tile([128, TPG, OUT_NODES], BF16,
                                         tag=f"c{g}")
                        r_b = r_g[:].unsqueeze(2).broadcast_to(
                            [128, TPG, OUT_NODES])
                        nc.vector.tensor_tensor(c_g[:], e_g[:], r_b,
                                                op=ALU.mult)
                        c_b = (c_g[:].unsqueeze(3)
                               .broadcast_to([128, TPG, OUT_NODES, OUT_DIM]))
                        nc.vector.tensor_tensor(
                            Wc_sb[:, gs, :].rearrange(
                                "p t (j d) -> p t j d", d=OUT_DIM),
                            W_sb[:, gs, :].rearrange(
                                "p t (j d) -> p t j d", d=OUT_DIM),
                            c_b, op=ALU.mult,
                        )
                        # next iteration's s-matmul links for this g
                        for t in range(g * TPG, (g + 1) * TPG):
                            for b0 in range(2):
                                nc.tensor.matmul(
                                    s_ps[:, b0, 0:JD],
                                    xT_sb[:, t * B + b0 * 128:
                                          t * B + b0 * 128 + 128],
                                    Wc_sb[:, t, :],
                                    start=(t == 0),
                                    stop=(t == NT - 1),
                                )
                else:
                    # ---- final iter: ReduceScatter, squash own slice ----
                    HB = 128 // N_CORES              # 16 rows per core
                    sout_rs = dramp.tile([HB, 2 * JD], CC_DT,
                                         tag="cc_out_rs")
                    nc.gpsimd.collective_compute(
                        "ReduceScatter", ALU.add, replica_groups=RG,
                        ins=[sin[:]], outs=[sout_rs[:]],
                    )
                    sl = workp.tile([HB, 2, JD], CC_DT, tag="sl")
                    nc.sync.dma_start(
                        sl[:], sout_rs[:].rearrange("p (g j) -> p g j",
                                                    j=JD))
                    ssq_l = workp.tile([HB, 2, JD], F32, tag="ssq_l")
                    nc.vector.tensor_tensor(ssq_l[:], sl[:], sl[:],
                                            op=ALU.mult)
                    sq_l = workp.tile([HB, 2, OUT_NODES], F32, tag="sq_l")
                    nc.vector.tensor_reduce(
                        sq_l[:],
                        ssq_l[:].rearrange("p g (j d) -> p g j d",
                                           d=OUT_DIM),
                        axis=mybir.AxisListType.X, op=ALU.add,
                    )
                    rt_l = workp.tile([HB, 2, OUT_NODES], F32, tag="rt_l")
                    nc.scalar.activation(rt_l[:], sq_l[:], AF.Sqrt)
                    den_l = workp.tile([HB, 2, OUT_NODES], F32,
                                       tag="den_l")
                    nc.vector.tensor_scalar_add(den_l[:], sq_l[:], 1.0)
                    dri_l = workp.tile([HB, 2, OUT_NODES], F32,
                                       tag="dri_l")
                    nc.vector.reciprocal(dri_l[:], den_l[:])
                    f_l = workp.tile([HB, 2, OUT_NODES], F32, tag="f_l")
                    nc.vector.tensor_tensor(f_l[:], rt_l[:], dri_l[:],
                                            op=ALU.mult)
                    v_l = workp.tile([HB, 2, JD], F32, tag="v_l")
                    f_lb = (f_l[:].unsqueeze(3)
                            .broadcast_to([HB, 2, OUT_NODES, OUT_DIM]))
                    nc.vector.tensor_tensor(
                        v_l[:].rearrange("p g (j d) -> p g j d",
                                         d=OUT_DIM),
                        sl[:].rearrange("p g (j d) -> p g j d", d=OUT_DIM),
                        f_lb, op=ALU.mult,
                    )
                    nc.sync.dma_start(
                        out_d[:].rearrange("(g p) j -> p g j", p=HB),
                        v_l[:])

    nc.compile()
    return nc


def make_inmaps(x, W):
    x = np.ascontiguousarray(np.asarray(x, dtype=np.float32))
    W = np.ascontiguousarray(np.asarray(W, dtype=np.float32))
    bf = ml_dtypes.bfloat16
    # 16 8x8 blocks of 1/B on the diagonal
    ones_blk = (np.kron(np.eye(128 // IN_DIM, dtype=np.float32),
                        np.ones((IN_DIM, IN_DIM), dtype=np.float32)) / B)
    in_maps = []
    for cid in range(N_CORES):
        sh = slice(cid * I_LOC, (cid + 1) * I_LOC)
        x_sh = x[:, sh, :].reshape(B, IK)
        xT = (np.ascontiguousarray(x_sh.T).reshape(NT, 128, B)
              .transpose(1, 0, 2))                    # [128, NT, B]
        xb = (np.ascontiguousarray(x_sh).reshape(2, 128, IK)
              .transpose(1, 0, 2))                    # [128, 2, IK]
        wb = (W[sh].transpose(0, 3, 1, 2).reshape(NT, 128, JD)
              .transpose(1, 0, 2))                    # [128, NT, JD]
        in_maps.append({
            "xT": np.ascontiguousarray(xT).astype(bf),
            "xb": np.ascontiguousarray(xb).astype(bf),
            "wb": np.ascontiguousarray(wb).astype(bf),
            "onesb": ones_blk.astype(bf),
        })
    return in_maps


def assemble_output(per_core_outs):
    HB = 128 // N_CORES
    v = np.zeros((B, OUT_NODES, OUT_DIM), dtype=np.float32)
    for c in range(N_CORES):
        o = per_core_outs[c]["out"].reshape(2, HB, OUT_NODES, OUT_DIM)
        v[16 * c:16 * c + HB] = o[0]
        v[128 + 16 * c:128 + 16 * c + HB] = o[1]
    return v[..., None].astype(np.float32)      # (256, 10, 16, 1)


_CACHED_NC = None


def kernel(x=None, W=None, **kw):
    global _CACHED_NC
    if x is None:
        x = kw["x"]
    if W is None:
        W = kw["W"]
    if _CACHED_NC is None:
        _CACHED_NC = build_nc()
    in_maps = make_inmaps(x, W)
    res = run_bass_kernel_spmd(
        _CACHED_NC, in_maps, core_ids=list(range(N_CORES)))
    return assemble_output(res.results)


if __name__ == "__main__":
    nc = build_nc()
    print("build + compile OK")
